# revision 1
# baseline (speedup 1.0000x reference)
"""Trainium2 Bass kernel for a GPT causal-attention block.

Problem: y = proj(causal_attention(x @ W_attn)), B=4, T=2048, C=1024, 16 heads.
Sharding: 8 cores = 4 batches x 2 head-groups (8 heads each). Each core
computes its batch's attention for its 8 heads plus the partial projection
(W_proj rows of its heads); the host sums the two partials per batch.

Per-core dataflow (fully transposed to keep every matmul at full PE rate):
  x -> xT (PE transpose)  ->  QT/KT [d, t], V'-packed [t, d-with-ones]
  ST chunk [kt=128, q=512] = KT_chunk.T @ QT   (contraction d=64)
  P = exp(ST/8) with causal mask via gpsimd affine_select (no max-sub needed:
      scores ~ N(0,1) after scaling, exp cannot overflow)
  yT[d, q] (+ denominator row from the ones-column) = V'.T @ P  (psum accum)
  reciprocal + K=1 broadcast matmul -> scale yT
  out[t, c] = yT_chunk.T @ Wp_rows  (psum accum over d-chunks)
All matmuls use float32r (TF32-like, 1 cyc/row at N=512): ~1.5e-4 rel error.
"""
import sys, os, contextlib

for _p in ("/opt/trn_rl_repo", "/root/.axon_site/_ro/trn_rl_repo"):
    if os.path.isdir(_p) and _p not in sys.path:
        sys.path.insert(0, _p)

import numpy as np

T, C, NHEAD, HS = 2048, 1024, 16, 64
NCORES = 8
HPC = NHEAD // 2          # heads per core = 8
DPC = HPC * HS            # head dims per core = 512
NCC = C // 128            # contraction chunks = 8
NQT = T // 512            # q tiles = 4
NCH = T // 128            # kt chunks = 16
NPAIR = HPC // 2          # head pairs per core = 4
NQUAD = T // 512          # t quads for transpose/qkv = 4
VLAG = 6                  # S->V chunk pipelining lag

_CACHE = {}


def _build():
    import concourse.tile as tile
    import concourse.bass as bass
    from concourse import bacc, mybir

    f32 = mybir.dt.float32
    f32r = mybir.dt.float32r
    FT = mybir.ActivationFunctionType
    from concourse.masks import make_identity

    nc = bacc.Bacc("TRN2", target_bir_lowering=False)
    x_d = nc.declare_dram_parameter("x", [T, C], f32, isOutput=False)
    wq_d = nc.declare_dram_parameter("wq", [C, DPC], f32, isOutput=False)
    wk_d = nc.declare_dram_parameter("wk", [C, DPC], f32, isOutput=False)
    wv_d = nc.declare_dram_parameter("wv", [C, DPC], f32, isOutput=False)
    wp_d = nc.declare_dram_parameter("wp", [DPC, C], f32, isOutput=False)
    ones_d = nc.declare_dram_parameter("ones_c", [128, 64], f32, isOutput=False)
    masks_d = nc.declare_dram_parameter("masks_c", [128, 4, 512], f32, isOutput=False)
    vones_d = nc.declare_dram_parameter("vones_c", [128, NPAIR, 33], f32, isOutput=False)
    o_d = nc.declare_dram_parameter("o", [T, C], f32, isOutput=True)

    x_r = x_d[:].rearrange("(n p) c -> n p c", p=128)     # [16, 128, 1024]
    o_r = o_d[:].rearrange("(n p) c -> n p c", p=128)

    with tile.TileContext(nc) as tc:
      with contextlib.ExitStack() as top:
        top.enter_context(nc.allow_low_precision(reason="f32r is 4-byte fp32"))
        const = top.enter_context(tc.tile_pool(name="const", bufs=1))
        persist = top.enter_context(tc.tile_pool(name="persist", bufs=1))

        ident = const.tile([128, 128], f32, tag="ident")
        make_identity(nc, ident)
        ones_row = const.tile([128, 64], f32, tag="ones_row")
        nc.sync.dma_start(out=ones_row, in_=ones_d[:])

        # persistent products of phase B
        qT = [persist.tile([128, T], f32r, tag=f"qT{u}", name=f"qT{u}") for u in range(NPAIR)]
        kT = [persist.tile([128, T], f32r, tag=f"kT{u}", name=f"kT{u}") for u in range(NPAIR)]
        vp = [persist.tile([128, NPAIR, 161], f32r, tag=f"vp{t}", name=f"vp{t}") for t in range(NCH)]

        # ---------------- phase A+B: xT quads -> QT/KT/V' ----------------
        with contextlib.ExitStack() as ab:
            wpool = ab.enter_context(tc.tile_pool(name="wpool", bufs=1))
            xpool = ab.enter_context(tc.tile_pool(name="xpool", bufs=6))
            xtq_pool = ab.enter_context(tc.tile_pool(name="xtq", bufs=1))
            psab = ab.enter_context(tc.tile_pool(name="psab", bufs=6, space="PSUM"))

            wq_sb = wpool.tile([128, NCC, DPC], f32r, tag="wq")
            wk_sb = wpool.tile([128, NCC, DPC], f32r, tag="wk")
            wv_sb = wpool.tile([128, NCC, DPC], f32r, tag="wv")
            nc.sync.dma_start(out=wq_sb, in_=wq_d[:].rearrange("(n p) d -> p n d", p=128).bitcast(f32r))
            nc.sync.dma_start(out=wk_sb, in_=wk_d[:].rearrange("(n p) d -> p n d", p=128).bitcast(f32r))
            nc.sync.dma_start(out=wv_sb, in_=wv_d[:].rearrange("(n p) d -> p n d", p=128).bitcast(f32r))
            for t in range(NCH):
                nc.sync.dma_start(out=vp[t][:, :, 64:97], in_=vones_d[:].bitcast(f32r))

            for q in range(NQUAD):
                x_nat = []
                for j in range(4):
                    xn = xpool.tile([128, C], f32, tag="x_nat")
                    nc.sync.dma_start(out=xn, in_=x_r[4 * q + j])
                    x_nat.append(xn)
                xTq = [xtq_pool.tile([128, 512], f32r, tag=f"xTq{cc}", name=f"xTq{cc}") for cc in range(NCC)]
                for cc in range(NCC):
                    pst = psab.tile([128, 512], f32, tag="mm")
                    for j in range(4):
                        nc.tensor.transpose(pst[:, j * 128:(j + 1) * 128],
                                            x_nat[j][:, cc * 128:(cc + 1) * 128], ident)
                    nc.vector.tensor_copy(xTq[cc][:], pst[:].bitcast(f32r))
                qs = slice(512 * q, 512 * (q + 1))
                # QT / KT for this t-range
                for dt in range(4):
                    psq = psab.tile([128, 512], f32, tag="mm")
                    for cc in range(NCC):
                        nc.tensor.matmul(psq[:], wq_sb[:, cc, dt * 128:(dt + 1) * 128],
                                         xTq[cc][:], start=(cc == 0), stop=(cc == NCC - 1))
                    nc.scalar.copy(out=qT[dt][:, qs], in_=psq[:].bitcast(f32r))
                    psk = psab.tile([128, 512], f32, tag="mm")
                    for cc in range(NCC):
                        nc.tensor.matmul(psk[:], wk_sb[:, cc, dt * 128:(dt + 1) * 128],
                                         xTq[cc][:], start=(cc == 0), stop=(cc == NCC - 1))
                    nc.scalar.copy(out=kT[dt][:, qs], in_=psk[:].bitcast(f32r))
                # V natural for the 4 t-chunks of this quad
                for j in range(4):
                    tt = 4 * q + j
                    psv = psab.tile([128, 512], f32, tag="mm")
                    for cc in range(NCC):
                        nc.tensor.matmul(psv[:], xTq[cc][:, j * 128:(j + 1) * 128],
                                         wv_sb[:, cc, :], start=(cc == 0), stop=(cc == NCC - 1))
                    # psv cols: head h at 64h; even heads -> vp[:, u, 0:64], odd -> vp[:, u, 97:161]
                    pv = psv.rearrange("p (u two d) -> p u two d", u=NPAIR, two=2)
                    nc.vector.tensor_copy(vp[tt][:, :, 0:64], pv[:, :, 0, :].bitcast(f32r))
                    nc.vector.tensor_copy(vp[tt][:, :, 97:161], pv[:, :, 1, :].bitcast(f32r))

        # ---------------- phases C+D ----------------
        with contextlib.ExitStack() as cd:
          ytpool = cd.enter_context(tc.tile_pool(name="ytpool", bufs=1))
          yT = [ytpool.tile([128, T], f32r, tag=f"yT{u}", name=f"yT{u}") for u in range(NPAIR)]
          # ---------------- phase C: attention ----------------
          with contextlib.ExitStack() as cs:
            ppool = cs.enter_context(tc.tile_pool(name="ppool", bufs=VLAG + 2))
            mpool = cs.enter_context(tc.tile_pool(name="mpool", bufs=1))
            masks = mpool.tile([128, 4, 512], f32, tag="masks")
            nc.sync.dma_start(out=masks, in_=masks_d[:])
            rpool = cs.enter_context(tc.tile_pool(name="rpool", bufs=3))
            pss = cs.enter_context(tc.tile_pool(name="pss", bufs=3, space="PSUM"))
            psy = cs.enter_context(tc.tile_pool(name="psy", bufs=2, space="PSUM"))
            psr = cs.enter_context(tc.tile_pool(name="psr", bufs=2, space="PSUM"))

            for u in range(NPAIR):
                for i in range(NQT):
                    L = 4 * (i + 1)
                    qs = slice(512 * i, 512 * (i + 1))
                    rb = psr.tile([128, 512], f32, tag="rb")
                    for h2 in range(2):
                        odd = h2 == 1
                        base = 64 * h2
                        ps_y = psy.tile([128, 512], f32, tag="ps_y")
                        r_sb = rpool.tile([128, 512], f32, tag="r_sb")

                        def vmm(c, _ps_y=ps_y, _u=u, _odd=odd, _L=L):
                            lhsT = vp[c][:, _u, 33:161] if _odd else vp[c][:, _u, 0:65]
                            out_ap = _ps_y[:, :] if _odd else _ps_y[0:65, :]
                            nc.tensor.matmul(out_ap, lhsT, P_tiles[c][:],
                                             start=(c == 0), stop=(c == _L - 1))

                        P_tiles = {}
                        for c in range(L):
                            s_ps = pss.tile([128, 512], f32, tag="s_ps")
                            nc.tensor.matmul(s_ps[:],
                                             kT[u][base:base + 64, c * 128:(c + 1) * 128],
                                             qT[u][base:base + 64, qs],
                                             start=True, stop=True)
                            P = ppool.tile([128, 512], f32r, tag="P")
                            nc.scalar.activation(out=P[:], in_=s_ps[:], func=FT.Exp,
                                                 scale=float(HS) ** -0.5)
                            if c >= 4 * i:
                                nc.vector.tensor_mul(P[:], P[:],
                                                     masks[:, c - 4 * i, :].bitcast(f32r))
                            P_tiles[c] = P
                            if c >= VLAG:
                                vmm(c - VLAG)
                        for c in range(max(0, L - VLAG), L):
                            vmm(c)
                        drow = 32 if odd else 64
                        nc.vector.reciprocal(out=r_sb[drow:drow + 1, :],
                                             in_=ps_y[drow:drow + 1, :])
                        nc.tensor.matmul(rb[base:base + 64, :],
                                         ones_row[drow:drow + 1, :],
                                         r_sb[drow:drow + 1, :], start=True, stop=True)
                        src = ps_y[64:128, :] if odd else ps_y[0:64, :]
                        nc.vector.tensor_copy(yT[u][base:base + 64, qs], src.bitcast(f32r))
                    nc.vector.tensor_mul(yT[u][:, qs], yT[u][:, qs], rb[:].bitcast(f32r))

          # ---------------- phase D: projection ----------------
          with contextlib.ExitStack() as ds:
            dpool = ds.enter_context(tc.tile_pool(name="dpool", bufs=1))
            opool = ds.enter_context(tc.tile_pool(name="opool", bufs=3))
            psd = ds.enter_context(tc.tile_pool(name="psd", bufs=4, space="PSUM"))
            wp_sb = dpool.tile([128, NPAIR, C], f32r, tag="wp")
            nc.sync.dma_start(out=wp_sb, in_=wp_d[:].rearrange("(n p) c -> p n c", p=128).bitcast(f32r))
            for tt in range(NCH):
                out_sb = opool.tile([128, C], f32, tag="out_sb")
                for ct in range(2):
                    po = psd.tile([128, 512], f32, tag="mm")
                    for u in range(NPAIR):
                        nc.tensor.matmul(po[:], yT[u][:, tt * 128:(tt + 1) * 128],
                                         wp_sb[:, u, ct * 512:(ct + 1) * 512],
                                         start=(u == 0), stop=(u == NPAIR - 1))
                    nc.vector.tensor_copy(out_sb[:, ct * 512:(ct + 1) * 512], po[:])
                nc.sync.dma_start(out=o_r[tt], in_=out_sb)

    nc.compile()
    return nc


def _get_nc():
    if "nc" not in _CACHE:
        _CACHE["nc"] = _build()
    return _CACHE["nc"]


def _in_maps(x, W_attn, W_proj):
    ones_c = np.ones((128, 64), np.float32)
    a_idx = np.arange(128)[:, None]
    b_idx = np.arange(512)[None, :]
    masks_c = np.stack([(b_idx - a_idx - 128 * j >= 0) for j in range(4)], 0)
    masks_c = np.ascontiguousarray(masks_c.transpose(1, 0, 2).astype(np.float32))
    vones_c = np.zeros((128, NPAIR, 33), np.float32)
    vones_c[:, :, 0:2] = 1.0
    maps = []
    for core in range(NCORES):
        b, g = core // 2, core % 2
        cs = slice(DPC * g, DPC * (g + 1))
        maps.append({
            "x": np.ascontiguousarray(x[b]),
            "wq": np.ascontiguousarray(W_attn[:, cs]),
            "wk": np.ascontiguousarray(W_attn[:, C:][:, cs]),
            "wv": np.ascontiguousarray(W_attn[:, 2 * C:][:, cs]),
            "wp": np.ascontiguousarray(W_proj[cs, :]),
            "ones_c": ones_c,
            "masks_c": masks_c,
            "vones_c": vones_c,
        })
    return maps


def _install_ntff_shim():
    """Provide antenv.axon_hooks (absent in this image) so trace=True works."""
    import sys as _sys, types, ctypes, contextlib as _cl
    if "antenv.axon_hooks" in _sys.modules:
        return
    so_path = "/opt/axon/libaxon_pjrt.so"
    try:
        lib = ctypes.CDLL(so_path)
        lib.axon_start_nrt_profile.argtypes = [ctypes.POINTER(ctypes.c_int64), ctypes.c_size_t]
        lib.axon_start_nrt_profile.restype = ctypes.c_int64
        lib.axon_stop_nrt_profile.argtypes = [ctypes.c_char_p]
        lib.axon_stop_nrt_profile.restype = ctypes.c_int64
    except (OSError, AttributeError):
        return

    @_cl.contextmanager
    def _hook(output_dir, device_ids):
        import jax
        jax.devices()
        if device_ids:
            ids = (ctypes.c_int64 * len(device_ids))(*device_ids)
            rc = lib.axon_start_nrt_profile(ids, len(device_ids))
        else:
            rc = lib.axon_start_nrt_profile(None, 0)
        if rc != 0:
            raise RuntimeError(f"axon_start_nrt_profile rc={rc}")
        try:
            yield
        finally:
            n = lib.axon_stop_nrt_profile(str(output_dir).encode())
            if n < 0:
                raise RuntimeError(f"axon_stop_nrt_profile rc={n}")

    mod = types.ModuleType("antenv.axon_hooks")
    mod.get_axon_ntff_profile_hook = lambda: _hook
    mod.set_axon_ntff_profile_hook = lambda h: None
    _sys.modules["antenv.axon_hooks"] = mod


def kernel(x, W_attn, W_proj, _trace=False):
    from concourse.bass_utils import run_bass_kernel_spmd
    if _trace:
        _install_ntff_shim()
    x = np.asarray(x, dtype=np.float32)
    W_attn = np.asarray(W_attn, dtype=np.float32)
    W_proj = np.asarray(W_proj, dtype=np.float32)
    nc = _get_nc()
    res = run_bass_kernel_spmd(nc, _in_maps(x, W_attn, W_proj),
                               core_ids=list(range(NCORES)), trace=_trace)
    out = np.empty((4, T, C), np.float32)
    for b in range(4):
        out[b] = res.results[2 * b]["o"] + res.results[2 * b + 1]["o"]
    if _trace:
        return out, res
    return out



# revision 21
# speedup vs baseline: 1.1865x; 1.1865x over previous
"""Trainium2 Bass kernel for a GPT causal-attention block.

Problem: y = proj(causal_attention(x @ W_attn)), B=4, T=2048, C=1024, 16 heads.
Sharding: 8 cores = 4 batches x 2 head-groups (8 heads each). Each core
computes its batch's attention for its 8 heads plus the partial projection
(W_proj rows of its heads); the host sums the two partials per batch.

v3 = baseline structure (proven stable on this HW) + targeted upgrades:
  - x/wq/wk/wv and qT/kT/vp/P in bf16 (1 cyc/row at any N; rel-err budget
    2e-2, bf16 end-to-end measures ~6e-3).
  - xT loaded straight from DRAM via DMA-transpose XBAR - no PE transposes.
  - reciprocal_approx_fast replaces the 3.3us/call DVE reciprocal.
  - yT stays f32r and the whole softmax tail (recip -> ones-broadcast matmul
    -> scale) is bit-identical in structure to the baseline, which runs
    reliably; the exotic v2 tail produced flaky device aborts.
Layout tricks kept from baseline: scores computed transposed (S^T chunks
[kt=128, q=512]), V packed with ones-columns so the softmax denominator
falls out of the AV matmul for free.
"""
import sys, os, contextlib

for _p in ("/opt/trn_rl_repo", "/root/.axon_site/_ro/trn_rl_repo"):
    if os.path.isdir(_p) and _p not in sys.path:
        sys.path.insert(0, _p)

import numpy as np

T, C, NHEAD, HS = 2048, 1024, 16, 64
NCORES = 8
HPC = NHEAD // 2          # heads per core = 8
DPC = HPC * HS            # head dims per core = 512
NCC = C // 128            # contraction chunks = 8
NQT = T // 512            # q tiles = 4
NCH = T // 128            # kt chunks = 16
NPAIR = HPC // 2          # head pairs per core = 4
NQUAD = T // 512          # t quads for qkv = 4
VLAG = 6                  # S->V chunk pipelining lag

_CACHE = {}


def _build():
    import concourse.tile as tile
    import concourse.bass as bass
    from concourse import bacc, mybir

    f32 = mybir.dt.float32
    f32r = mybir.dt.float32r
    bf16 = mybir.dt.bfloat16
    FT = mybir.ActivationFunctionType

    nc = bacc.Bacc("TRN2", target_bir_lowering=False)
    x_d = nc.declare_dram_parameter("x", [T, C], bf16, isOutput=False)
    wq_d = nc.declare_dram_parameter("wq", [C, DPC], bf16, isOutput=False)
    wk_d = nc.declare_dram_parameter("wk", [C, DPC], bf16, isOutput=False)
    wv_d = nc.declare_dram_parameter("wv", [C, DPC], bf16, isOutput=False)
    wp_d = nc.declare_dram_parameter("wp", [DPC, C], f32, isOutput=False)
    ones_d = nc.declare_dram_parameter("ones_c", [128, 64], f32, isOutput=False)
    masks_d = nc.declare_dram_parameter("masks_c", [128, 4, 512], bf16, isOutput=False)
    vones_d = nc.declare_dram_parameter("vones_c", [128, NPAIR, 64], bf16, isOutput=False)
    o_d = nc.declare_dram_parameter("o", [T, C], f32, isOutput=True)

    x_ap = x_d[:]
    o_r = o_d[:].rearrange("(n p) c -> n p c", p=128)

    with tile.TileContext(nc) as tc:
      with contextlib.ExitStack() as top:
        top.enter_context(nc.allow_low_precision(reason="bf16 within rel-err budget"))
        const = top.enter_context(tc.tile_pool(name="const", bufs=1))
        persist = top.enter_context(tc.tile_pool(name="persist", bufs=1))

        ones_row = const.tile([128, 64], f32, tag="ones_row")
        nc.sync.dma_start(out=ones_row, in_=ones_d[:])

        # persistent products
        qT = [persist.tile([128, T], bf16, tag=f"qT{u}", name=f"qT{u}") for u in range(NPAIR)]
        kT = [persist.tile([128, T], bf16, tag=f"kT{u}", name=f"kT{u}") for u in range(NPAIR)]
        vp = [persist.tile([128, NPAIR, 256], bf16, tag=f"vp{t}", name=f"vp{t}") for t in range(NCH)]
        xT = [persist.tile([128, T], bf16, tag=f"xT{cc}", name=f"xT{cc}") for cc in range(NCC)]

        # ---------------- phase A+B: DMA-transposed x -> QT/KT/V' ----------------
        with contextlib.ExitStack() as ab:
            wpool = ab.enter_context(tc.tile_pool(name="wpool", bufs=1))
            psab = ab.enter_context(tc.tile_pool(name="psab", bufs=6, space="PSUM"))

            wq_sb = wpool.tile([128, NCC, DPC], bf16, tag="wq")
            wk_sb = wpool.tile([128, NCC, DPC], bf16, tag="wk")
            wv_sb = wpool.tile([128, NCC, DPC], bf16, tag="wv")
            nc.sync.dma_start(out=wq_sb, in_=wq_d[:].rearrange("(n p) d -> p n d", p=128))
            nc.sync.dma_start(out=wk_sb, in_=wk_d[:].rearrange("(n p) d -> p n d", p=128))
            nc.sync.dma_start(out=wv_sb, in_=wv_d[:].rearrange("(n p) d -> p n d", p=128))
            for t in range(NCH):
                nc.sync.dma_start(out=vp[t][:, :, 0:64], in_=vones_d[:])
                nc.sync.dma_start(out=vp[t][:, :, 128:192], in_=vones_d[:])
            for q in range(NQUAD):
                for cc in range(NCC):
                    nc.sync.dma_start(
                        out=xT[cc][:, 512 * q:512 * (q + 1)],
                        in_=x_ap[512 * q:512 * (q + 1), 128 * cc:128 * (cc + 1)],
                        transpose=True)

            for q in range(NQUAD):
                qs = slice(512 * q, 512 * (q + 1))
                # QT / KT for this t-range
                for dt in range(4):
                    psq = psab.tile([128, 512], f32, tag="mm")
                    for cc in range(NCC):
                        nc.tensor.matmul(psq[:], wq_sb[:, cc, dt * 128:(dt + 1) * 128],
                                         xT[cc][:, qs], start=(cc == 0), stop=(cc == NCC - 1))
                    nc.scalar.copy(out=qT[dt][:, qs], in_=psq[:])
                    psk = psab.tile([128, 512], f32, tag="mm")
                    for cc in range(NCC):
                        nc.tensor.matmul(psk[:], wk_sb[:, cc, dt * 128:(dt + 1) * 128],
                                         xT[cc][:, qs], start=(cc == 0), stop=(cc == NCC - 1))
                    nc.scalar.copy(out=kT[dt][:, qs], in_=psk[:])
                # V natural for the 4 t-chunks of this quad
                for j in range(4):
                    tt = 4 * q + j
                    psv = psab.tile([128, 512], f32, tag="mm")
                    for cc in range(NCC):
                        nc.tensor.matmul(psv[:], xT[cc][:, tt * 128:(tt + 1) * 128],
                                         wv_sb[:, cc, :], start=(cc == 0), stop=(cc == NCC - 1))
                    # psv cols: head h at 64h; even heads -> vp[:, u, 0:64], odd -> vp[:, u, 97:161]
                    pv = psv.rearrange("p (u two d) -> p u two d", u=NPAIR, two=2)
                    nc.vector.tensor_copy(vp[tt][:, :, 64:128], pv[:, :, 0, :])
                    nc.vector.tensor_copy(vp[tt][:, :, 192:256], pv[:, :, 1, :])

        # ---------------- phases C+D ----------------
        with contextlib.ExitStack() as cd:
          ytpool = cd.enter_context(tc.tile_pool(name="ytpool", bufs=1))
          yT = [ytpool.tile([128, T], f32r, tag=f"yT{u}", name=f"yT{u}") for u in range(NPAIR)]
          # ---------------- phase C: attention ----------------
          with contextlib.ExitStack() as cs:
            ppool = cs.enter_context(tc.tile_pool(name="ppool", bufs=VLAG + 2))
            mpool = cs.enter_context(tc.tile_pool(name="mpool", bufs=1))
            masks = mpool.tile([128, 4, 512], bf16, tag="masks")
            nc.sync.dma_start(out=masks, in_=masks_d[:])
            rpool = cs.enter_context(tc.tile_pool(name="rpool", bufs=3))
            pss = cs.enter_context(tc.tile_pool(name="pss", bufs=3, space="PSUM"))
            psy = cs.enter_context(tc.tile_pool(name="psy", bufs=2, space="PSUM"))
            psr = cs.enter_context(tc.tile_pool(name="psr", bufs=2, space="PSUM"))

            for u in range(NPAIR):
                for i in range(NQT):
                    L = 4 * (i + 1)
                    qs = slice(512 * i, 512 * (i + 1))
                    rb = psr.tile([128, 512], f32, tag="rb")
                    for h2 in range(2):
                        odd = h2 == 1
                        base = 64 * h2
                        ps_y = psy.tile([128, 512], f32, tag="ps_y")
                        r_sb = rpool.tile([128, 512], f32, tag="r_sb")

                        def vmm(c, _ps_y=ps_y, _u=u, _odd=odd, _L=L):
                            lhsT = vp[c][:, _u, 128:256] if _odd else vp[c][:, _u, 0:128]
                            nc.tensor.matmul(_ps_y[:, :], lhsT, P_tiles[c][:],
                                             start=(c == 0), stop=(c == _L - 1))

                        P_tiles = {}
                        for c in range(L):
                            s_ps = pss.tile([128, 512], f32, tag="s_ps")
                            nc.tensor.matmul(s_ps[:],
                                             kT[u][base:base + 64, c * 128:(c + 1) * 128],
                                             qT[u][base:base + 64, qs],
                                             start=True, stop=True)
                            P = ppool.tile([128, 512], bf16, tag="P")
                            nc.scalar.activation(out=P[:], in_=s_ps[:], func=FT.Exp,
                                                 scale=float(HS) ** -0.5)
                            if c >= 4 * i:
                                nc.vector.tensor_mul(P[:], P[:],
                                                     masks[:, c - 4 * i, :])
                            P_tiles[c] = P
                            if c >= VLAG:
                                vmm(c - VLAG)
                        for c in range(max(0, L - VLAG), L):
                            vmm(c)
                        nc.vector.reciprocal_approx_fast(out=r_sb[0:1, :],
                                                         in_=ps_y[0:1, :])
                        nc.tensor.matmul(rb[base:base + 64, :],
                                         ones_row[0:1, :],
                                         r_sb[0:1, :], start=True, stop=True)
                        nc.vector.tensor_copy(yT[u][base:base + 64, qs],
                                              ps_y[64:128, :].bitcast(f32r))
                    nc.vector.tensor_mul(yT[u][:, qs], yT[u][:, qs], rb[:].bitcast(f32r))

          # ---------------- phase D: projection ----------------
          with contextlib.ExitStack() as ds:
            dpool = ds.enter_context(tc.tile_pool(name="dpool", bufs=1))
            opool = ds.enter_context(tc.tile_pool(name="opool", bufs=3))
            psd = ds.enter_context(tc.tile_pool(name="psd", bufs=4, space="PSUM"))
            wp_sb = dpool.tile([128, NPAIR, C], f32r, tag="wp")
            nc.sync.dma_start(out=wp_sb, in_=wp_d[:].rearrange("(n p) c -> p n c", p=128).bitcast(f32r))
            for tt in range(NCH):
                out_sb = opool.tile([128, C], f32, tag="out_sb")
                for ct in range(2):
                    po = psd.tile([128, 512], f32, tag="mm")
                    for u in range(NPAIR):
                        nc.tensor.matmul(po[:], yT[u][:, tt * 128:(tt + 1) * 128],
                                         wp_sb[:, u, ct * 512:(ct + 1) * 512],
                                         start=(u == 0), stop=(u == NPAIR - 1))
                    nc.vector.tensor_copy(out_sb[:, ct * 512:(ct + 1) * 512], po[:])
                nc.sync.dma_start(out=o_r[tt], in_=out_sb)

    nc.compile()
    return nc


def _get_nc():
    if "nc" not in _CACHE:
        _CACHE["nc"] = _build()
    return _CACHE["nc"]


def _in_maps(x, W_attn, W_proj):
    import ml_dtypes
    bf = ml_dtypes.bfloat16
    ones_c = np.ones((128, 64), np.float32)
    a_idx = np.arange(128)[:, None]
    b_idx = np.arange(512)[None, :]
    masks_c = np.stack([(b_idx - a_idx - 128 * j >= 0) for j in range(4)], 0)
    masks_c = np.ascontiguousarray(masks_c.transpose(1, 0, 2)).astype(bf)
    vones_c = np.zeros((128, NPAIR, 64), bf)
    vones_c[:, :, 0] = 1.0
    maps = []
    for core in range(NCORES):
        b, g = core // 2, core % 2
        cs = slice(DPC * g, DPC * (g + 1))
        maps.append({
            "x": np.ascontiguousarray(x[b]).astype(bf),
            "wq": np.ascontiguousarray(W_attn[:, cs]).astype(bf),
            "wk": np.ascontiguousarray(W_attn[:, C:][:, cs]).astype(bf),
            "wv": np.ascontiguousarray(W_attn[:, 2 * C:][:, cs]).astype(bf),
            "wp": np.ascontiguousarray(W_proj[cs, :]),
            "ones_c": ones_c,
            "masks_c": masks_c,
            "vones_c": vones_c,
        })
    return maps


def _install_ntff_shim():
    """Provide antenv.axon_hooks (absent in this image) so trace=True works."""
    import sys as _sys, types, ctypes, contextlib as _cl
    if "antenv.axon_hooks" in _sys.modules:
        return
    so_path = "/opt/axon/libaxon_pjrt.so"
    try:
        lib = ctypes.CDLL(so_path)
        lib.axon_start_nrt_profile.argtypes = [ctypes.POINTER(ctypes.c_int64), ctypes.c_size_t]
        lib.axon_start_nrt_profile.restype = ctypes.c_int64
        lib.axon_stop_nrt_profile.argtypes = [ctypes.c_char_p]
        lib.axon_stop_nrt_profile.restype = ctypes.c_int64
    except (OSError, AttributeError):
        return

    @_cl.contextmanager
    def _hook(output_dir, device_ids):
        import jax
        jax.devices()
        if device_ids:
            ids = (ctypes.c_int64 * len(device_ids))(*device_ids)
            rc = lib.axon_start_nrt_profile(ids, len(device_ids))
        else:
            rc = lib.axon_start_nrt_profile(None, 0)
        if rc != 0:
            raise RuntimeError(f"axon_start_nrt_profile rc={rc}")
        try:
            yield
        finally:
            n = lib.axon_stop_nrt_profile(str(output_dir).encode())
            if n < 0:
                raise RuntimeError(f"axon_stop_nrt_profile rc={n}")

    mod = types.ModuleType("antenv.axon_hooks")
    mod.get_axon_ntff_profile_hook = lambda: _hook
    mod.set_axon_ntff_profile_hook = lambda h: None
    _sys.modules["antenv.axon_hooks"] = mod


def kernel(x, W_attn, W_proj, _trace=False):
    from concourse.bass_utils import run_bass_kernel_spmd
    if _trace:
        _install_ntff_shim()
    x = np.asarray(x, dtype=np.float32)
    W_attn = np.asarray(W_attn, dtype=np.float32)
    W_proj = np.asarray(W_proj, dtype=np.float32)
    nc = _get_nc()
    res = run_bass_kernel_spmd(nc, _in_maps(x, W_attn, W_proj),
                               core_ids=list(range(NCORES)), trace=_trace)
    out = np.empty((4, T, C), np.float32)
    for b in range(4):
        out[b] = res.results[2 * b]["o"] + res.results[2 * b + 1]["o"]
    if _trace:
        return out, res
    return out


# revision 22
# speedup vs baseline: 1.8097x; 1.5253x over previous
"""Trainium2 Bass kernel for a GPT causal-attention block.

Problem: y = proj(causal_attention(x @ W_attn)), B=4, T=2048, C=1024, 16 heads.
Sharding: 8 cores = 4 batches x 2 head-groups (8 heads each). Each core
computes its batch's attention for its 8 heads plus the partial projection
(W_proj rows of its heads); the host sums the two partials per batch.

v3 = baseline structure (proven stable on this HW) + targeted upgrades:
  - x/wq/wk/wv and qT/kT/vp/P in bf16 (1 cyc/row at any N; rel-err budget
    2e-2, bf16 end-to-end measures ~6e-3).
  - xT loaded straight from DRAM via DMA-transpose XBAR - no PE transposes.
  - reciprocal_approx_fast replaces the 3.3us/call DVE reciprocal.
  - yT stays f32r and the whole softmax tail (recip -> ones-broadcast matmul
    -> scale) is bit-identical in structure to the baseline, which runs
    reliably; the exotic v2 tail produced flaky device aborts.
Layout tricks kept from baseline: scores computed transposed (S^T chunks
[kt=128, q=512]), V packed with ones-columns so the softmax denominator
falls out of the AV matmul for free.
"""
import sys, os, contextlib

for _p in ("/opt/trn_rl_repo", "/root/.axon_site/_ro/trn_rl_repo"):
    if os.path.isdir(_p) and _p not in sys.path:
        sys.path.insert(0, _p)

import numpy as np

T, C, NHEAD, HS = 2048, 1024, 16, 64
NCORES = 8
HPC = NHEAD // 2          # heads per core = 8
DPC = HPC * HS            # head dims per core = 512
NCC = C // 128            # contraction chunks = 8
NQT = T // 512            # q tiles = 4
NCH = T // 128            # kt chunks = 16
NPAIR = HPC // 2          # head pairs per core = 4
NQUAD = T // 512          # t quads for qkv = 4
VLAG = 6                  # S->V chunk pipelining lag

_CACHE = {}


def _build():
    import concourse.tile as tile
    import concourse.bass as bass
    from concourse import bacc, mybir

    f32 = mybir.dt.float32
    f32r = mybir.dt.float32r
    bf16 = mybir.dt.bfloat16
    FT = mybir.ActivationFunctionType

    nc = bacc.Bacc("TRN2", target_bir_lowering=False)
    x_d = nc.declare_dram_parameter("x", [T, C], bf16, isOutput=False)
    wq_d = nc.declare_dram_parameter("wq", [C, DPC], bf16, isOutput=False)
    wk_d = nc.declare_dram_parameter("wk", [C, DPC], bf16, isOutput=False)
    wv_d = nc.declare_dram_parameter("wv", [C, DPC], bf16, isOutput=False)
    wp_d = nc.declare_dram_parameter("wp", [DPC, C], f32, isOutput=False)
    ones_d = nc.declare_dram_parameter("ones_c", [128, 64], bf16, isOutput=False)
    masks_d = nc.declare_dram_parameter("masks_c", [128, 4, 512], bf16, isOutput=False)
    vones_d = nc.declare_dram_parameter("vones_c", [128, NPAIR, 64], bf16, isOutput=False)
    o_d = nc.declare_dram_parameter("o", [T, C], f32, isOutput=True)

    x_ap = x_d[:]
    o_r = o_d[:].rearrange("(n p) c -> n p c", p=128)

    with tile.TileContext(nc) as tc:
      with contextlib.ExitStack() as top:
        top.enter_context(nc.allow_low_precision(reason="bf16 within rel-err budget"))
        const = top.enter_context(tc.tile_pool(name="const", bufs=1))
        persist = top.enter_context(tc.tile_pool(name="persist", bufs=1))

        ones_row = const.tile([128, 64], bf16, tag="ones_row")
        nc.sync.dma_start(out=ones_row, in_=ones_d[:])

        # persistent products
        qT = [persist.tile([128, T], bf16, tag=f"qT{u}", name=f"qT{u}") for u in range(NPAIR)]
        kT = [persist.tile([128, T], bf16, tag=f"kT{u}", name=f"kT{u}") for u in range(NPAIR)]
        vp = [persist.tile([128, NPAIR, 256], bf16, tag=f"vp{t}", name=f"vp{t}") for t in range(NCH)]
        xT = [persist.tile([128, T], bf16, tag=f"xT{cc}", name=f"xT{cc}") for cc in range(NCC)]

        # ---------------- phase A+B: DMA-transposed x -> QT/KT/V' ----------------
        with contextlib.ExitStack() as ab:
            wpool = ab.enter_context(tc.tile_pool(name="wpool", bufs=1))
            psab = ab.enter_context(tc.tile_pool(name="psab", bufs=6, space="PSUM"))

            wq_sb = wpool.tile([128, NCC, DPC], bf16, tag="wq")
            wk_sb = wpool.tile([128, NCC, DPC], bf16, tag="wk")
            wv_sb = wpool.tile([128, NCC, DPC], bf16, tag="wv")
            def xt_dma(q):
                for cc in range(NCC):
                    nc.sync.dma_start(
                        out=xT[cc][:, 512 * q:512 * (q + 1)],
                        in_=x_ap[512 * q:512 * (q + 1), 128 * cc:128 * (cc + 1)],
                        transpose=True)
            nc.sync.dma_start(out=wq_sb, in_=wq_d[:].rearrange("(n p) d -> p n d", p=128))
            xt_dma(0)
            nc.sync.dma_start(out=wk_sb, in_=wk_d[:].rearrange("(n p) d -> p n d", p=128))
            xt_dma(1)
            nc.sync.dma_start(out=wv_sb, in_=wv_d[:].rearrange("(n p) d -> p n d", p=128))
            xt_dma(2)
            xt_dma(3)
            for t in range(NCH):
                nc.sync.dma_start(out=vp[t][:, :, 0:64], in_=vones_d[:])
                nc.sync.dma_start(out=vp[t][:, :, 128:192], in_=vones_d[:])

            for q in range(NQUAD):
                qs = slice(512 * q, 512 * (q + 1))
                # QT / KT for this t-range
                for dt in range(4):
                    psq = psab.tile([128, 512], f32, tag="mm")
                    for cc in range(NCC):
                        nc.tensor.matmul(psq[:], wq_sb[:, cc, dt * 128:(dt + 1) * 128],
                                         xT[cc][:, qs], start=(cc == 0), stop=(cc == NCC - 1))
                    nc.scalar.copy(out=qT[dt][:, qs], in_=psq[:])
                    psk = psab.tile([128, 512], f32, tag="mm")
                    for cc in range(NCC):
                        nc.tensor.matmul(psk[:], wk_sb[:, cc, dt * 128:(dt + 1) * 128],
                                         xT[cc][:, qs], start=(cc == 0), stop=(cc == NCC - 1))
                    nc.scalar.copy(out=kT[dt][:, qs], in_=psk[:])
            # V natural, all t-chunks (after QK so the first QK starts sooner)
            for tt in range(NCH):
                if True:
                    psv = psab.tile([128, 512], f32, tag="mm")
                    for cc in range(NCC):
                        nc.tensor.matmul(psv[:], xT[cc][:, tt * 128:(tt + 1) * 128],
                                         wv_sb[:, cc, :], start=(cc == 0), stop=(cc == NCC - 1))
                    # psv cols: head h at 64h; even heads -> vp[:, u, 0:64], odd -> vp[:, u, 97:161]
                    pv = psv.rearrange("p (u two d) -> p u two d", u=NPAIR, two=2)
                    nc.vector.tensor_copy(vp[tt][:, :, 64:128], pv[:, :, 0, :])
                    nc.vector.tensor_copy(vp[tt][:, :, 192:256], pv[:, :, 1, :])

        # ---------------- phases C+D ----------------
        with contextlib.ExitStack() as cd:
          ytpool = cd.enter_context(tc.tile_pool(name="ytpool", bufs=1))
          yT = [ytpool.tile([128, T], f32r, tag=f"yT{u}", name=f"yT{u}") for u in range(NPAIR)]
          # ---------------- phase C: attention ----------------
          with contextlib.ExitStack() as cs:
            ppool = cs.enter_context(tc.tile_pool(name="ppool", bufs=VLAG + 2))
            mpool = cs.enter_context(tc.tile_pool(name="mpool", bufs=1))
            masks = mpool.tile([128, 4, 512], bf16, tag="masks")
            nc.sync.dma_start(out=masks, in_=masks_d[:])
            rpool = cs.enter_context(tc.tile_pool(name="rpool", bufs=3))
            pss = cs.enter_context(tc.tile_pool(name="pss", bufs=3, space="PSUM"))
            psy = cs.enter_context(tc.tile_pool(name="psy", bufs=2, space="PSUM"))
            psr = cs.enter_context(tc.tile_pool(name="psr", bufs=2, space="PSUM"))

            for u in range(NPAIR):
                for i in range(NQT):
                    L = 4 * (i + 1)
                    qs = slice(512 * i, 512 * (i + 1))
                    rb = psr.tile([128, 512], f32, tag="rb")
                    for h2 in range(2):
                        odd = h2 == 1
                        base = 64 * h2
                        ps_y = psy.tile([128, 512], f32, tag="ps_y")
                        r_sb = rpool.tile([128, 512], f32, tag="r_sb")
                        r_bf = rpool.tile([128, 512], bf16, tag="r_bf")

                        def vmm(c, _ps_y=ps_y, _u=u, _odd=odd, _L=L):
                            lhsT = vp[c][:, _u, 128:256] if _odd else vp[c][:, _u, 0:128]
                            nc.tensor.matmul(_ps_y[:, :], lhsT, P_tiles[c][:],
                                             start=(c == 0), stop=(c == _L - 1))

                        P_tiles = {}
                        for c in range(L):
                            s_ps = pss.tile([128, 512], f32, tag="s_ps")
                            nc.tensor.matmul(s_ps[:],
                                             kT[u][base:base + 64, c * 128:(c + 1) * 128],
                                             qT[u][base:base + 64, qs],
                                             start=True, stop=True)
                            P = ppool.tile([128, 512], bf16, tag="P")
                            nc.scalar.activation(out=P[:], in_=s_ps[:], func=FT.Exp,
                                                 scale=float(HS) ** -0.5)
                            if c >= 4 * i:
                                nc.vector.tensor_mul(P[:], P[:],
                                                     masks[:, c - 4 * i, :])
                            P_tiles[c] = P
                            if c >= VLAG:
                                vmm(c - VLAG)
                        for c in range(max(0, L - VLAG), L):
                            vmm(c)
                        nc.vector.reciprocal_approx_fast(out=r_sb[0:1, :],
                                                         in_=ps_y[0:1, :])
                        nc.vector.tensor_copy(r_bf[0:1, :], r_sb[0:1, :])
                        nc.tensor.matmul(rb[base:base + 64, :],
                                         ones_row[0:1, :],
                                         r_bf[0:1, :], start=True, stop=True)
                        nc.vector.tensor_copy(yT[u][base:base + 64, qs],
                                              ps_y[64:128, :].bitcast(f32r))
                    nc.vector.tensor_mul(yT[u][:, qs], yT[u][:, qs], rb[:].bitcast(f32r))

          # ---------------- phase D: projection ----------------
          with contextlib.ExitStack() as ds:
            dpool = ds.enter_context(tc.tile_pool(name="dpool", bufs=1))
            opool = ds.enter_context(tc.tile_pool(name="opool", bufs=3))
            psd = ds.enter_context(tc.tile_pool(name="psd", bufs=4, space="PSUM"))
            wp_sb = dpool.tile([128, NPAIR, C], f32r, tag="wp")
            nc.sync.dma_start(out=wp_sb, in_=wp_d[:].rearrange("(n p) c -> p n c", p=128).bitcast(f32r))
            for tt in range(NCH):
                out_sb = opool.tile([128, C], f32, tag="out_sb")
                for ct in range(2):
                    po = psd.tile([128, 512], f32, tag="mm")
                    for u in range(NPAIR):
                        nc.tensor.matmul(po[:], yT[u][:, tt * 128:(tt + 1) * 128],
                                         wp_sb[:, u, ct * 512:(ct + 1) * 512],
                                         start=(u == 0), stop=(u == NPAIR - 1))
                    nc.vector.tensor_copy(out_sb[:, ct * 512:(ct + 1) * 512], po[:])
                nc.sync.dma_start(out=o_r[tt], in_=out_sb)

    nc.compile()
    return nc


def _get_nc():
    if "nc" not in _CACHE:
        _CACHE["nc"] = _build()
    return _CACHE["nc"]


def _in_maps(x, W_attn, W_proj):
    import ml_dtypes
    bf = ml_dtypes.bfloat16
    ones_c = np.ones((128, 64), ml_dtypes.bfloat16)
    a_idx = np.arange(128)[:, None]
    b_idx = np.arange(512)[None, :]
    masks_c = np.stack([(b_idx - a_idx - 128 * j >= 0) for j in range(4)], 0)
    masks_c = np.ascontiguousarray(masks_c.transpose(1, 0, 2)).astype(bf)
    vones_c = np.zeros((128, NPAIR, 64), bf)
    vones_c[:, :, 0] = 1.0
    maps = []
    for core in range(NCORES):
        b, g = core // 2, core % 2
        cs = slice(DPC * g, DPC * (g + 1))
        maps.append({
            "x": np.ascontiguousarray(x[b]).astype(bf),
            "wq": np.ascontiguousarray(W_attn[:, cs]).astype(bf),
            "wk": np.ascontiguousarray(W_attn[:, C:][:, cs]).astype(bf),
            "wv": np.ascontiguousarray(W_attn[:, 2 * C:][:, cs]).astype(bf),
            "wp": np.ascontiguousarray(W_proj[cs, :]),
            "ones_c": ones_c,
            "masks_c": masks_c,
            "vones_c": vones_c,
        })
    return maps


def _install_ntff_shim():
    """Provide antenv.axon_hooks (absent in this image) so trace=True works."""
    import sys as _sys, types, ctypes, contextlib as _cl
    if "antenv.axon_hooks" in _sys.modules:
        return
    so_path = "/opt/axon/libaxon_pjrt.so"
    try:
        lib = ctypes.CDLL(so_path)
        lib.axon_start_nrt_profile.argtypes = [ctypes.POINTER(ctypes.c_int64), ctypes.c_size_t]
        lib.axon_start_nrt_profile.restype = ctypes.c_int64
        lib.axon_stop_nrt_profile.argtypes = [ctypes.c_char_p]
        lib.axon_stop_nrt_profile.restype = ctypes.c_int64
    except (OSError, AttributeError):
        return

    @_cl.contextmanager
    def _hook(output_dir, device_ids):
        import jax
        jax.devices()
        if device_ids:
            ids = (ctypes.c_int64 * len(device_ids))(*device_ids)
            rc = lib.axon_start_nrt_profile(ids, len(device_ids))
        else:
            rc = lib.axon_start_nrt_profile(None, 0)
        if rc != 0:
            raise RuntimeError(f"axon_start_nrt_profile rc={rc}")
        try:
            yield
        finally:
            n = lib.axon_stop_nrt_profile(str(output_dir).encode())
            if n < 0:
                raise RuntimeError(f"axon_stop_nrt_profile rc={n}")

    mod = types.ModuleType("antenv.axon_hooks")
    mod.get_axon_ntff_profile_hook = lambda: _hook
    mod.set_axon_ntff_profile_hook = lambda h: None
    _sys.modules["antenv.axon_hooks"] = mod


def kernel(x, W_attn, W_proj, _trace=False):
    from concourse.bass_utils import run_bass_kernel_spmd
    if _trace:
        _install_ntff_shim()
    x = np.asarray(x, dtype=np.float32)
    W_attn = np.asarray(W_attn, dtype=np.float32)
    W_proj = np.asarray(W_proj, dtype=np.float32)
    nc = _get_nc()
    res = run_bass_kernel_spmd(nc, _in_maps(x, W_attn, W_proj),
                               core_ids=list(range(NCORES)), trace=_trace)
    out = np.empty((4, T, C), np.float32)
    for b in range(4):
        out[b] = res.results[2 * b]["o"] + res.results[2 * b + 1]["o"]
    if _trace:
        return out, res
    return out


# revision 23
# speedup vs baseline: 1.9202x; 1.0610x over previous
"""Trainium2 Bass kernel for a GPT causal-attention block.

Problem: y = proj(causal_attention(x @ W_attn)), B=4, T=2048, C=1024, 16 heads.
Sharding: 8 cores = 4 batches x 2 head-groups (8 heads each). Each core
computes its batch's attention for its 8 heads plus the partial projection
(W_proj rows of its heads); the host sums the two partials per batch.

v3 = baseline structure (proven stable on this HW) + targeted upgrades:
  - x/wq/wk/wv and qT/kT/vp/P in bf16 (1 cyc/row at any N; rel-err budget
    2e-2, bf16 end-to-end measures ~6e-3).
  - xT loaded straight from DRAM via DMA-transpose XBAR - no PE transposes.
  - reciprocal_approx_fast replaces the 3.3us/call DVE reciprocal.
  - yT stays f32r and the whole softmax tail (recip -> ones-broadcast matmul
    -> scale) is bit-identical in structure to the baseline, which runs
    reliably; the exotic v2 tail produced flaky device aborts.
Layout tricks kept from baseline: scores computed transposed (S^T chunks
[kt=128, q=512]), V packed with ones-columns so the softmax denominator
falls out of the AV matmul for free.
"""
import sys, os, contextlib

for _p in ("/opt/trn_rl_repo", "/root/.axon_site/_ro/trn_rl_repo"):
    if os.path.isdir(_p) and _p not in sys.path:
        sys.path.insert(0, _p)

import numpy as np

T, C, NHEAD, HS = 2048, 1024, 16, 64
NCORES = 8
HPC = NHEAD // 2          # heads per core = 8
DPC = HPC * HS            # head dims per core = 512
NCC = C // 128            # contraction chunks = 8
NQT = T // 512            # q tiles = 4
NCH = T // 128            # kt chunks = 16
NPAIR = HPC // 2          # head pairs per core = 4
NQUAD = T // 512          # t quads for qkv = 4
VLAG = 6                  # S->V chunk pipelining lag

_CACHE = {}


def _build():
    import concourse.tile as tile
    import concourse.bass as bass
    from concourse import bacc, mybir

    f32 = mybir.dt.float32
    f32r = mybir.dt.float32r
    bf16 = mybir.dt.bfloat16
    FT = mybir.ActivationFunctionType

    nc = bacc.Bacc("TRN2", target_bir_lowering=False)
    xt_d = nc.declare_dram_parameter("xt", [C, T], bf16, isOutput=False)
    wq_d = nc.declare_dram_parameter("wq", [C, DPC], bf16, isOutput=False)
    wk_d = nc.declare_dram_parameter("wk", [C, DPC], bf16, isOutput=False)
    wv_d = nc.declare_dram_parameter("wv", [C, DPC], bf16, isOutput=False)
    wp_d = nc.declare_dram_parameter("wp", [DPC, C], f32, isOutput=False)
    ones_d = nc.declare_dram_parameter("ones_c", [128, 64], bf16, isOutput=False)
    masks_d = nc.declare_dram_parameter("masks_c", [128, 4, 512], bf16, isOutput=False)
    vones_d = nc.declare_dram_parameter("vones_c", [128, NPAIR, 64], bf16, isOutput=False)
    o_d = nc.declare_dram_parameter("o", [T, C], f32, isOutput=True)

    xt_ap = xt_d[:]
    o_r = o_d[:].rearrange("(n p) c -> n p c", p=128)

    with tile.TileContext(nc) as tc:
      with contextlib.ExitStack() as top:
        top.enter_context(nc.allow_low_precision(reason="bf16 within rel-err budget"))
        const = top.enter_context(tc.tile_pool(name="const", bufs=1))
        persist = top.enter_context(tc.tile_pool(name="persist", bufs=1))

        ones_row = const.tile([128, 64], bf16, tag="ones_row")
        nc.sync.dma_start(out=ones_row, in_=ones_d[:])

        # persistent products
        qT = [persist.tile([128, T], bf16, tag=f"qT{u}", name=f"qT{u}") for u in range(NPAIR)]
        kT = [persist.tile([128, T], bf16, tag=f"kT{u}", name=f"kT{u}") for u in range(NPAIR)]
        vp = [persist.tile([128, NPAIR, 256], bf16, tag=f"vp{t}", name=f"vp{t}") for t in range(NCH)]
        xT = [persist.tile([128, T], bf16, tag=f"xT{cc}", name=f"xT{cc}") for cc in range(NCC)]

        # ---------------- phase A+B: DMA-transposed x -> QT/KT/V' ----------------
        with contextlib.ExitStack() as ab:
            wpool = ab.enter_context(tc.tile_pool(name="wpool", bufs=1))
            psab = ab.enter_context(tc.tile_pool(name="psab", bufs=6, space="PSUM"))

            wq_sb = wpool.tile([128, NCC, DPC], bf16, tag="wq")
            wk_sb = wpool.tile([128, NCC, DPC], bf16, tag="wk")
            wv_sb = wpool.tile([128, NCC, DPC], bf16, tag="wv")
            def xt_dma(q):
                for cc in range(NCC):
                    nc.sync.dma_start(
                        out=xT[cc][:, 512 * q:512 * (q + 1)],
                        in_=xt_ap[128 * cc:128 * (cc + 1), 512 * q:512 * (q + 1)])
            xt_dma(0)
            nc.sync.dma_start(out=wq_sb, in_=wq_d[:].rearrange("(n p) d -> p n d", p=128))
            nc.sync.dma_start(out=wk_sb, in_=wk_d[:].rearrange("(n p) d -> p n d", p=128))
            xt_dma(1)
            nc.sync.dma_start(out=wv_sb, in_=wv_d[:].rearrange("(n p) d -> p n d", p=128))
            xt_dma(2)
            xt_dma(3)
            for t in range(NCH):
                nc.sync.dma_start(out=vp[t][:, :, 0:64], in_=vones_d[:])
                nc.sync.dma_start(out=vp[t][:, :, 128:192], in_=vones_d[:])

            for q in range(NQUAD):
                qs = slice(512 * q, 512 * (q + 1))
                # QT / KT for this t-range
                for dt in range(4):
                    psq = psab.tile([128, 512], f32, tag="mm")
                    for cc in range(NCC):
                        nc.tensor.matmul(psq[:], wq_sb[:, cc, dt * 128:(dt + 1) * 128],
                                         xT[cc][:, qs], start=(cc == 0), stop=(cc == NCC - 1))
                    nc.scalar.copy(out=qT[dt][:, qs], in_=psq[:])
                    psk = psab.tile([128, 512], f32, tag="mm")
                    for cc in range(NCC):
                        nc.tensor.matmul(psk[:], wk_sb[:, cc, dt * 128:(dt + 1) * 128],
                                         xT[cc][:, qs], start=(cc == 0), stop=(cc == NCC - 1))
                    nc.scalar.copy(out=kT[dt][:, qs], in_=psk[:])
            # V natural, all t-chunks (after QK so the first QK starts sooner)
            for tt in range(NCH):
                if True:
                    psv = psab.tile([128, 512], f32, tag="mm")
                    for cc in range(NCC):
                        nc.tensor.matmul(psv[:], xT[cc][:, tt * 128:(tt + 1) * 128],
                                         wv_sb[:, cc, :], start=(cc == 0), stop=(cc == NCC - 1))
                    # psv cols: head h at 64h; even heads -> vp[:, u, 0:64], odd -> vp[:, u, 97:161]
                    pv = psv.rearrange("p (u two d) -> p u two d", u=NPAIR, two=2)
                    nc.vector.tensor_copy(vp[tt][:, :, 64:128], pv[:, :, 0, :])
                    nc.vector.tensor_copy(vp[tt][:, :, 192:256], pv[:, :, 1, :])

        # ---------------- phases C+D ----------------
        with contextlib.ExitStack() as cd:
          ytpool = cd.enter_context(tc.tile_pool(name="ytpool", bufs=1))
          yT = [ytpool.tile([128, T], f32r, tag=f"yT{u}", name=f"yT{u}") for u in range(NPAIR)]
          dpool = cd.enter_context(tc.tile_pool(name="dpool", bufs=1))
          wp_sb = dpool.tile([128, NPAIR, C], f32r, tag="wp")
          nc.sync.dma_start(out=wp_sb, in_=wp_d[:].rearrange("(n p) c -> p n c", p=128).bitcast(f32r))
          # ---------------- phase C: attention ----------------
          with contextlib.ExitStack() as cs:
            ppool = cs.enter_context(tc.tile_pool(name="ppool", bufs=VLAG + 2))
            mpool = cs.enter_context(tc.tile_pool(name="mpool", bufs=1))
            masks = mpool.tile([128, 4, 512], bf16, tag="masks")
            nc.sync.dma_start(out=masks, in_=masks_d[:])
            rpool = cs.enter_context(tc.tile_pool(name="rpool", bufs=3))
            pss = cs.enter_context(tc.tile_pool(name="pss", bufs=3, space="PSUM"))
            psy = cs.enter_context(tc.tile_pool(name="psy", bufs=2, space="PSUM"))
            psr = cs.enter_context(tc.tile_pool(name="psr", bufs=2, space="PSUM"))

            for u in range(NPAIR):
                for i in range(NQT):
                    L = 4 * (i + 1)
                    qs = slice(512 * i, 512 * (i + 1))
                    rb = psr.tile([128, 512], f32, tag="rb")
                    for h2 in range(2):
                        odd = h2 == 1
                        base = 64 * h2
                        ps_y = psy.tile([128, 512], f32, tag="ps_y")
                        r_sb = rpool.tile([128, 512], f32, tag="r_sb")
                        r_bf = rpool.tile([128, 512], bf16, tag="r_bf")

                        def vmm(c, _ps_y=ps_y, _u=u, _odd=odd, _L=L):
                            lhsT = vp[c][:, _u, 128:256] if _odd else vp[c][:, _u, 0:128]
                            nc.tensor.matmul(_ps_y[:, :], lhsT, P_tiles[c][:],
                                             start=(c == 0), stop=(c == _L - 1))

                        P_tiles = {}
                        for c in range(L):
                            s_ps = pss.tile([128, 512], f32, tag="s_ps")
                            nc.tensor.matmul(s_ps[:],
                                             kT[u][base:base + 64, c * 128:(c + 1) * 128],
                                             qT[u][base:base + 64, qs],
                                             start=True, stop=True)
                            P = ppool.tile([128, 512], bf16, tag="P")
                            nc.scalar.activation(out=P[:], in_=s_ps[:], func=FT.Exp,
                                                 scale=float(HS) ** -0.5)
                            if c >= 4 * i:
                                nc.vector.tensor_mul(P[:], P[:],
                                                     masks[:, c - 4 * i, :])
                            P_tiles[c] = P
                            if c >= VLAG:
                                vmm(c - VLAG)
                        for c in range(max(0, L - VLAG), L):
                            vmm(c)
                        nc.vector.reciprocal_approx_fast(out=r_sb[0:1, :],
                                                         in_=ps_y[0:1, :])
                        nc.vector.tensor_copy(r_bf[0:1, :], r_sb[0:1, :])
                        nc.tensor.matmul(rb[base:base + 64, :],
                                         ones_row[0:1, :],
                                         r_bf[0:1, :], start=True, stop=True)
                        nc.vector.tensor_copy(yT[u][base:base + 64, qs],
                                              ps_y[64:128, :].bitcast(f32r))
                    nc.vector.tensor_mul(yT[u][:, qs], yT[u][:, qs], rb[:].bitcast(f32r))

          # ---------------- phase D: projection ----------------
          with contextlib.ExitStack() as ds:
            opool = ds.enter_context(tc.tile_pool(name="opool", bufs=3))
            psd = ds.enter_context(tc.tile_pool(name="psd", bufs=4, space="PSUM"))
            for tt in range(NCH):
                out_sb = opool.tile([128, C], f32, tag="out_sb")
                for ct in range(2):
                    po = psd.tile([128, 512], f32, tag="mm")
                    for u in range(NPAIR):
                        nc.tensor.matmul(po[:], yT[u][:, tt * 128:(tt + 1) * 128],
                                         wp_sb[:, u, ct * 512:(ct + 1) * 512],
                                         start=(u == 0), stop=(u == NPAIR - 1))
                    nc.vector.tensor_copy(out_sb[:, ct * 512:(ct + 1) * 512], po[:])
                nc.sync.dma_start(out=o_r[tt], in_=out_sb)

    nc.compile()
    return nc


def _get_nc():
    if "nc" not in _CACHE:
        _CACHE["nc"] = _build()
    return _CACHE["nc"]


def _in_maps(x, W_attn, W_proj):
    import ml_dtypes
    bf = ml_dtypes.bfloat16
    ones_c = np.ones((128, 64), ml_dtypes.bfloat16)
    a_idx = np.arange(128)[:, None]
    b_idx = np.arange(512)[None, :]
    masks_c = np.stack([(b_idx - a_idx - 128 * j >= 0) for j in range(4)], 0)
    masks_c = np.ascontiguousarray(masks_c.transpose(1, 0, 2)).astype(bf)
    vones_c = np.zeros((128, NPAIR, 64), bf)
    vones_c[:, :, 0] = 1.0
    maps = []
    for core in range(NCORES):
        b, g = core // 2, core % 2
        cs = slice(DPC * g, DPC * (g + 1))
        maps.append({
            "xt": np.ascontiguousarray(x[b].T).astype(bf),
            "wq": np.ascontiguousarray(W_attn[:, cs]).astype(bf),
            "wk": np.ascontiguousarray(W_attn[:, C:][:, cs]).astype(bf),
            "wv": np.ascontiguousarray(W_attn[:, 2 * C:][:, cs]).astype(bf),
            "wp": np.ascontiguousarray(W_proj[cs, :]),
            "ones_c": ones_c,
            "masks_c": masks_c,
            "vones_c": vones_c,
        })
    return maps


def _install_ntff_shim():
    """Provide antenv.axon_hooks (absent in this image) so trace=True works."""
    import sys as _sys, types, ctypes, contextlib as _cl
    if "antenv.axon_hooks" in _sys.modules:
        return
    so_path = "/opt/axon/libaxon_pjrt.so"
    try:
        lib = ctypes.CDLL(so_path)
        lib.axon_start_nrt_profile.argtypes = [ctypes.POINTER(ctypes.c_int64), ctypes.c_size_t]
        lib.axon_start_nrt_profile.restype = ctypes.c_int64
        lib.axon_stop_nrt_profile.argtypes = [ctypes.c_char_p]
        lib.axon_stop_nrt_profile.restype = ctypes.c_int64
    except (OSError, AttributeError):
        return

    @_cl.contextmanager
    def _hook(output_dir, device_ids):
        import jax
        jax.devices()
        if device_ids:
            ids = (ctypes.c_int64 * len(device_ids))(*device_ids)
            rc = lib.axon_start_nrt_profile(ids, len(device_ids))
        else:
            rc = lib.axon_start_nrt_profile(None, 0)
        if rc != 0:
            raise RuntimeError(f"axon_start_nrt_profile rc={rc}")
        try:
            yield
        finally:
            n = lib.axon_stop_nrt_profile(str(output_dir).encode())
            if n < 0:
                raise RuntimeError(f"axon_stop_nrt_profile rc={n}")

    mod = types.ModuleType("antenv.axon_hooks")
    mod.get_axon_ntff_profile_hook = lambda: _hook
    mod.set_axon_ntff_profile_hook = lambda h: None
    _sys.modules["antenv.axon_hooks"] = mod


def kernel(x, W_attn, W_proj, _trace=False):
    from concourse.bass_utils import run_bass_kernel_spmd
    if _trace:
        _install_ntff_shim()
    x = np.asarray(x, dtype=np.float32)
    W_attn = np.asarray(W_attn, dtype=np.float32)
    W_proj = np.asarray(W_proj, dtype=np.float32)
    nc = _get_nc()
    res = run_bass_kernel_spmd(nc, _in_maps(x, W_attn, W_proj),
                               core_ids=list(range(NCORES)), trace=_trace)
    out = np.empty((4, T, C), np.float32)
    for b in range(4):
        out[b] = res.results[2 * b]["o"] + res.results[2 * b + 1]["o"]
    if _trace:
        return out, res
    return out
